# revision 3
# baseline (speedup 1.0000x reference)
"""AttnBlock (GroupNorm -> QKV 1x1 conv -> spatial attention with softmax over
query-H axis -> output projection + residual) for B=8, C=128, H=W=48 on 8
Trainium2 NeuronCores, data-parallel over batch (1 batch per core).

Math per batch (N = H*W = 2304 spatial positions, C = 128 channels):
  xn = GroupNorm(x; 32 groups of 4 channels)
  q/k/v = W @ xn + b              (per-position 1x1 conv = C x C matmul)
  S[q', kp] = q[:,q'] . k[:,kp] / sqrt(C)
  attn = softmax over the query-H axis: for fixed (w, kp), normalize over h
  ctx[c, (h,w)] = sum_kp attn[(h,w), kp] * v[c, kp]
  out = x + Wo @ ctx + bo

Device layout notes:
  - Channels live on the 128 SBUF partitions; spatial positions on the free axis.
  - S is computed transposed (S^T [kp, q']) per 128-key chunk so the softmax
    reduction (over h) runs along the free axis on the Vector engine.
  - Queries are stored w-major (q' = w*48 + h) so each softmax group of 48 h
    values is contiguous; the reorder is free (permuted access pattern on the
    projection evacuation).
  - Matmul inputs use float32r (TF32-like, 1 cycle/row for N>=256).
"""

import sys

sys.path.insert(0, "/opt/trn_rl_repo")

import numpy as np

import concourse.bass as bass
import concourse.mybir as mybir
import concourse.tile as tile
from concourse import bacc, bass_utils

B, C, H, W = 8, 128, 48, 48
N = H * W  # 2304 spatial positions
GROUPS = 32
GSIZE = C // GROUPS  # 4 channels per group
EPS = 1e-5
NCORES = 8

F32 = mybir.dt.float32
F32R = mybir.dt.float32r
AF = mybir.ActivationFunctionType
OP = mybir.AluOpType

NCHUNK = N // 128  # 18 key chunks of 128 positions
QG = 768  # query-group width for S^T staging / exp granularity
NQG = N // QG  # 3
# ctx PSUM bank tiling of the 2304-wide context accumulator
CTX_SIZES = [512, 512, 512, 512, 256]
CTX_OFFS = [0, 512, 1024, 1536, 2048]


def _build_program():
    nc = bacc.Bacc("TRN2", target_bir_lowering=False, debug=False)

    dram = {}

    def din(name, shape, dt=F32):
        dram[name] = nc.dram_tensor(name, shape, dt, kind="ExternalInput")
        return dram[name]

    x_d = din("x", [C, N])
    gnw_d = din("gn_w", [C, 1])
    gnb_d = din("gn_b", [C, 1])
    wqT_d = din("wqT", [C, C], F32R)
    wkT_d = din("wkT", [C, C], F32R)
    wvT_d = din("wvT", [C, C], F32R)
    woT_d = din("woT", [C, C], F32R)
    bq_d = din("bq", [C, 1])
    bk_d = din("bk", [C, 1])
    bv_d = din("bv", [C, 1])
    bo_d = din("bo", [C, 1])
    gmat_d = din("gmat", [C, GROUPS], F32R)  # c -> group indicator
    gexp_d = din("gexp", [GROUPS, C], F32R)  # group -> c indicator
    ident_d = din("ident", [C, C], F32R)
    out_d = nc.dram_tensor("out", [C, N], F32, kind="ExternalOutput")

    with tile.TileContext(nc) as tc:
        with (
            tc.tile_pool(name="const", bufs=1) as const,
            tc.tile_pool(name="data", bufs=1) as data,
            tc.tile_pool(name="small", bufs=1) as small,
            tc.tile_pool(name="soft", bufs=2) as soft,
            tc.tile_pool(name="epool", bufs=2) as epool,
        ):
            # ---- constant / input loads ----
            wqT = const.tile([C, C], F32R)
            wkT = const.tile([C, C], F32R)
            wvT = const.tile([C, C], F32R)
            woT = const.tile([C, C], F32R)
            gmat = const.tile([C, GROUPS], F32R)
            gexp = const.tile([GROUPS, C], F32R)
            ident = const.tile([C, C], F32R)
            gnw = const.tile([C, 1], F32)
            gnb = const.tile([C, 1], F32)
            bq = const.tile([C, 1], F32)
            bk = const.tile([C, 1], F32)
            bv = const.tile([C, 1], F32)
            bo = const.tile([C, 1], F32)
            for t, d in [
                (wqT, wqT_d), (wkT, wkT_d), (wvT, wvT_d), (woT, woT_d),
                (gmat, gmat_d), (gexp, gexp_d), (ident, ident_d),
                (gnw, gnw_d), (gnb, gnb_d),
                (bq, bq_d), (bk, bk_d), (bv, bv_d), (bo, bo_d),
            ]:
                nc.sync.dma_start(t[:], d[:])

            tx = data.tile([C, N], F32)
            nc.sync.dma_start(tx[:], x_d[:])

            # ---- GroupNorm statistics ----
            # per-channel sum (DVE) and sum of squares (ACT, fused square+accum)
            stats_f = small.tile([C, 2], F32)
            sq_scratch = epool.tile([C, N], F32R, tag="E")
            nc.vector.tensor_reduce(
                stats_f[:, 0:1], tx[:], axis=mybir.AxisListType.X, op=OP.add
            )
            nc.scalar.activation(
                sq_scratch[:], tx[:], AF.Square, accum_out=stats_f[:, 1:2]
            )
            stats = small.tile([C, 2], F32R)
            nc.vector.tensor_copy(stats[:], stats_f[:])

            with tc.tile_pool(name="gnps", bufs=1, space="PSUM") as gnps:
                # group sums: [32, 2] = gmat^T @ stats
                psg = gnps.tile([GROUPS, 2], F32)
                nc.tensor.matmul(psg[:], gmat[:], stats[:], start=True, stop=True)

                inv_n = 1.0 / (GSIZE * N)
                t32 = small.tile([GROUPS, 4], F32)
                # mean, E[x^2]
                nc.vector.tensor_scalar_mul(t32[:, 0:1], psg[:, 0:1], inv_n)
                nc.vector.tensor_scalar_mul(t32[:, 1:2], psg[:, 1:2], inv_n)
                # var = E[x^2] - mean^2
                nc.vector.tensor_mul(t32[:, 2:3], t32[:, 0:1], t32[:, 0:1])
                nc.vector.tensor_sub(t32[:, 3:4], t32[:, 1:2], t32[:, 2:3])
                # rstd = 1/sqrt(var + eps)
                eps_t = small.tile([GROUPS, 1], F32)
                nc.vector.memset(eps_t[:], EPS)
                nc.scalar.activation(t32[:, 2:3], t32[:, 3:4], AF.Sqrt, bias=eps_t[:])
                rstd_f = small.tile([GROUPS, 1], F32)
                nc.vector.reciprocal(rstd_f[:], t32[:, 2:3])
                mstat = small.tile([GROUPS, 2], F32R)
                nc.vector.tensor_copy(mstat[:, 0:1], t32[:, 0:1])
                nc.vector.tensor_copy(mstat[:, 1:2], rstd_f[:])

                # expand group stats back to channels: [128, 2] = gexp^T @ mstat
                pse = gnps.tile([C, 2], F32)
                nc.tensor.matmul(pse[:], gexp[:], mstat[:], start=True, stop=True)

                # A = rstd_c * gn_w ; Bc = gn_b - mean_c * A
                A_sb = small.tile([C, 1], F32)
                B_sb = small.tile([C, 1], F32)
                nc.vector.tensor_mul(A_sb[:], pse[:, 1:2], gnw[:])
                nc.vector.tensor_mul(B_sb[:], pse[:, 0:1], A_sb[:])
                nc.vector.tensor_sub(B_sb[:], gnb[:], B_sb[:])

            # xn = x * A + Bc   (per-partition scale+bias on ACT)
            xn = data.tile([C, N], F32R)
            nc.scalar.activation(xn[:], tx[:], AF.Identity, bias=B_sb[:], scale=A_sb[:])

            # ---- Q/K/V projections ----
            # q is written w-major (q' = w*48 + h) via a permuted evacuation AP.
            q = data.tile([C, N], F32R)
            k = data.tile([C, N], F32R)
            v = data.tile([C, N], F32R)
            q_wh = q[:].rearrange("p (w h) -> p h w", h=H)  # dims (h s1, w s48)
            with tc.tile_pool(name="projps", bufs=2, space="PSUM") as projps:
                for wT, bias, dst, permute in (
                    (wqT, bq, q, True),
                    (wkT, bk, k, False),
                    (wvT, bv, v, False),
                ):
                    for g in range(NQG):
                        pp = projps.tile([C, QG], F32, tag="pp")
                        o = g * QG
                        nc.tensor.matmul(
                            pp[:, 0:512], wT[:], xn[:, o : o + 512],
                            start=True, stop=True,
                        )
                        nc.tensor.matmul(
                            pp[:, 512:QG], wT[:], xn[:, o + 512 : o + QG],
                            start=True, stop=True,
                        )
                        if permute:
                            # natural block g covers h in [16g, 16g+16), all w
                            outv = q_wh[:, 16 * g : 16 * (g + 1), :]
                            inv = pp[:].rearrange("p (h w) -> p h w", w=W)
                        else:
                            outv = dst[:, o : o + QG]
                            inv = pp[:, :]
                        nc.scalar.activation(outv, inv, AF.Identity, bias=bias[:])

                # ---- v^T chunks (for AV lhsT): PE transpose ----
                vT = data.tile([C, NCHUNK * C], F32R)
                for grp in range(0, NCHUNK, 4):
                    cnt = min(4, NCHUNK - grp)
                    pvt = projps.tile([C, 512], F32R, tag="pvt")
                    for j in range(cnt):
                        ch = grp + j
                        nc.tensor.transpose(
                            pvt[:, 128 * j : 128 * (j + 1)],
                            v[:, 128 * ch : 128 * (ch + 1)],
                            ident[:],
                        )
                    nc.scalar.copy(
                        vT[:, 128 * grp : 128 * (grp + cnt)], pvt[:, : 128 * cnt]
                    )

            # ---- main attention loop over key chunks ----
            with (
                tc.tile_pool(name="ctxps", bufs=1, space="PSUM") as ctxps,
                tc.tile_pool(name="sps", bufs=1, space="PSUM") as sps,
            ):
                ctx_ps = [
                    ctxps.tile([C, sz], F32, tag=f"ctx{i}", name=f"ctx_ps{i}")
                    for i, sz in enumerate(CTX_SIZES)
                ]
                e_tiles = [None] * NCHUNK

                def emit_av(ch):
                    ec = e_tiles[ch]
                    for i, (o, sz) in enumerate(zip(CTX_OFFS, CTX_SIZES)):
                        nc.tensor.matmul(
                            ctx_ps[i][:, :],
                            vT[:, 128 * ch : 128 * (ch + 1)],
                            ec[:, o : o + sz],
                            start=(ch == 0),
                            stop=(ch == NCHUNK - 1),
                        )

                for ch in range(NCHUNK):
                    ec = epool.tile([C, N], F32R, tag="E")
                    e_tiles[ch] = ec
                    klhs = k[:, 128 * ch : 128 * (ch + 1)]
                    for g in range(NQG):
                        ps = sps.tile([C, QG], F32, tag="spsum")
                        o = g * QG
                        nc.tensor.matmul(
                            ps[:, 0:512], klhs, q[:, o : o + 512],
                            start=True, stop=True,
                        )
                        nc.tensor.matmul(
                            ps[:, 512:QG], klhs, q[:, o + 512 : o + QG],
                            start=True, stop=True,
                        )
                        # exp straight out of PSUM into the E tile
                        nc.scalar.activation(ec[:, o : o + QG], ps[:, :], AF.Exp)
                        # interleave previous chunk's AV matmuls between the
                        # S^T groups so the PE FIFO never stalls on the exp
                        if ch > 0 and g == 0:
                            emit_av(ch - 1)

                    # softmax denominator over h (contiguous inner groups of 48)
                    dsum = soft.tile([C, W], F32, tag="D")
                    nc.vector.tensor_reduce(
                        dsum[:],
                        ec[:].rearrange("p (w h) -> p w h", h=H),
                        axis=mybir.AxisListType.X,
                        op=OP.add,
                    )
                    rden = soft.tile([C, W], F32, tag="R")
                    nc.vector.reciprocal(rden[:], dsum[:])
                    # normalize in place: E[p, w, h] *= rden[p, w]
                    nc.vector.tensor_tensor(
                        out=ec[:].rearrange("p (w h) -> p w h", h=H),
                        in0=ec[:].rearrange("p (w h) -> p w h", h=H),
                        in1=rden[:, :, None].to_broadcast([C, W, H]),
                        op=OP.mult,
                    )
                emit_av(NCHUNK - 1)

                # ---- evacuate ctx ----
                ctx_all = data.tile([C, N], F32R)
                for i, (o, sz) in enumerate(zip(CTX_OFFS, CTX_SIZES)):
                    nc.scalar.copy(ctx_all[:, o : o + sz], ctx_ps[i][:, :])

            # ---- output projection (un-permute back to natural order) + residual ----
            out_nat = data.tile([C, N], F32)
            out_wh = out_nat[:].rearrange("p (h w) -> p w h", w=W)  # dims (w s1, h s48)
            with tc.tile_pool(name="ops", bufs=2, space="PSUM") as ops:
                for g in range(NQG):
                    po = ops.tile([C, QG], F32, tag="po")
                    o = g * QG
                    nc.tensor.matmul(
                        po[:, 0:512], woT[:], ctx_all[:, o : o + 512],
                        start=True, stop=True,
                    )
                    nc.tensor.matmul(
                        po[:, 512:QG], woT[:], ctx_all[:, o + 512 : o + QG],
                        start=True, stop=True,
                    )
                    # w-major block g covers w in [16g, 16g+16), all h
                    nc.scalar.activation(
                        out_wh[:, 16 * g : 16 * (g + 1), :],
                        po[:].rearrange("p (w h) -> p w h", h=H),
                        AF.Identity,
                        bias=bo[:],
                    )
            nc.vector.tensor_add(out_nat[:], out_nat[:], tx[:])
            nc.sync.dma_start(out_d[:], out_nat[:])

    nc.compile()
    return nc


_PROGRAM_CACHE = None


def kernel(**inputs: np.ndarray) -> np.ndarray:
    global _PROGRAM_CACHE
    if _PROGRAM_CACHE is None:
        _PROGRAM_CACHE = _build_program()
    nc = _PROGRAM_CACHE

    f32 = lambda a: np.ascontiguousarray(np.asarray(a), dtype=np.float32)
    x = f32(inputs["x"])  # [B, C, H, W]
    scale = 1.0 / np.sqrt(np.float32(C))

    gmat = np.zeros((C, GROUPS), np.float32)
    gmat[np.arange(C), np.arange(C) // GSIZE] = 1.0
    gexp = np.ascontiguousarray(gmat.T)

    shared = {
        "gn_w": f32(inputs["gn_w"]).reshape(C, 1),
        "gn_b": f32(inputs["gn_b"]).reshape(C, 1),
        "wqT": np.ascontiguousarray(f32(inputs["wq"]).T * scale),
        "wkT": np.ascontiguousarray(f32(inputs["wk"]).T),
        "wvT": np.ascontiguousarray(f32(inputs["wv"]).T),
        "woT": np.ascontiguousarray(f32(inputs["wo"]).T),
        "bq": f32(inputs["bq"]).reshape(C, 1) * scale,
        "bk": f32(inputs["bk"]).reshape(C, 1),
        "bv": f32(inputs["bv"]).reshape(C, 1),
        "bo": f32(inputs["bo"]).reshape(C, 1),
        "gmat": gmat,
        "gexp": gexp,
        "ident": np.eye(C, dtype=np.float32),
    }
    in_maps = [
        {**shared, "x": np.ascontiguousarray(x[b].reshape(C, N))} for b in range(B)
    ]

    res = bass_utils.run_bass_kernel_spmd(nc, in_maps, core_ids=list(range(NCORES)))
    out = np.stack([res.results[b]["out"].reshape(C, H, W) for b in range(B)])
    return out.astype(inputs["x"].dtype if hasattr(inputs["x"], "dtype") else np.float32)


# revision 4
# speedup vs baseline: 1.1713x; 1.1713x over previous
"""AttnBlock (GroupNorm -> QKV 1x1 conv -> spatial attention with softmax over
query-H axis -> output projection + residual) for B=8, C=128, H=W=48 on 8
Trainium2 NeuronCores, data-parallel over batch (1 batch per core).

Math per batch (N = H*W = 2304 spatial positions, C = 128 channels):
  xn = GroupNorm(x; 32 groups of 4 channels)
  q/k/v = W @ xn + b              (per-position 1x1 conv = C x C matmul)
  S[q', kp] = q[:,q'] . k[:,kp] / sqrt(C)
  attn = softmax over the query-H axis: for fixed (w, kp), normalize over h
  ctx[c, (h,w)] = sum_kp attn[(h,w), kp] * v[c, kp]
  out = x + Wo @ ctx + bo

Device mapping:
  - Channels on the 128 SBUF partitions; spatial positions on the free axis.
  - S computed transposed (S^T [kp, q']) per 128-key chunk so the softmax
    reduction (over h) runs along the free axis (VectorE / grouped reduce).
  - Queries stored w-major (q' = w*48 + h) so each softmax group of 48 h
    values is contiguous; reorder is free (permuted APs on the projection
    evacuations).
  - QK^T in float32r (TF32-like); E/attn/V^T in bf16 for the second matmul.
  - The normalize multiplies are split between VectorE and GpSimd; the AV
    matmuls for chunk c are emitted two chunks later so the PE never waits
    on the softmax chain.
"""

import sys

sys.path.insert(0, "/opt/trn_rl_repo")

import numpy as np

import concourse.bass as bass
import concourse.mybir as mybir
import concourse.tile as tile
from concourse import bacc, bass_utils

B, C, H, W = 8, 128, 48, 48
N = H * W  # 2304
GROUPS = 32
GSIZE = C // GROUPS
EPS = 1e-5
NCORES = 8

F32 = mybir.dt.float32
F32R = mybir.dt.float32r
BF16 = mybir.dt.bfloat16
AF = mybir.ActivationFunctionType
OP = mybir.AluOpType

NCHUNK = N // 128  # 18 key chunks
QG = 768  # S^T staging / exp granularity
NQG = N // QG  # 3
CTX_SIZES = [512, 512, 512, 512, 256]
CTX_OFFS = [0, 512, 1024, 1536, 2048]
# chunks whose normalize-mul runs on GpSimd instead of VectorE
GP_MUL_EVERY = 2  # every other chunk


def _build_program():
    nc = bacc.Bacc("TRN2", target_bir_lowering=False, debug=False)

    def din(name, shape, dt=F32):
        return nc.dram_tensor(name, shape, dt, kind="ExternalInput")

    x_d = din("x", [C, N])
    gnw_d = din("gn_w", [C, 1])
    gnb_d = din("gn_b", [C, 1])
    wqT_d = din("wqT", [C, C], F32R)
    wkT_d = din("wkT", [C, C], F32R)
    wvT_d = din("wvT", [C, C], F32R)
    woT_d = din("woT", [C, C], F32R)
    bq_d = din("bq", [C, 1])
    bk_d = din("bk", [C, 1])
    bv_d = din("bv", [C, 1])
    bo_d = din("bo", [C, 1])
    gmat_d = din("gmat", [C, GROUPS], F32R)
    gexp_d = din("gexp", [GROUPS, C], F32R)
    ident_d = din("ident", [C, C], BF16)
    out_d = nc.dram_tensor("out", [C, N], F32, kind="ExternalOutput")

    with tile.TileContext(nc) as tc:
        with (
            tc.tile_pool(name="const", bufs=1) as const,
            tc.tile_pool(name="data", bufs=1) as data,
            tc.tile_pool(name="small", bufs=1) as small,
            tc.tile_pool(name="soft", bufs=3) as soft,
            tc.tile_pool(name="epool", bufs=3) as epool,
        ):
            # ---- constant / input loads ----
            wqT = const.tile([C, C], F32R)
            wkT = const.tile([C, C], F32R)
            wvT = const.tile([C, C], F32R)
            woT = const.tile([C, C], F32R)
            gmat = const.tile([C, GROUPS], F32R)
            gexp = const.tile([GROUPS, C], F32R)
            ident = const.tile([C, C], BF16)
            gnw = const.tile([C, 1], F32)
            gnb = const.tile([C, 1], F32)
            bq = const.tile([C, 1], F32)
            bk = const.tile([C, 1], F32)
            bv = const.tile([C, 1], F32)
            bo = const.tile([C, 1], F32)
            for t, d in [
                (wqT, wqT_d), (wkT, wkT_d), (wvT, wvT_d), (woT, woT_d),
                (gmat, gmat_d), (gexp, gexp_d), (ident, ident_d),
                (gnw, gnw_d), (gnb, gnb_d),
                (bq, bq_d), (bk, bk_d), (bv, bv_d), (bo, bo_d),
            ]:
                nc.sync.dma_start(t[:], d[:])

            tx = data.tile([C, N], F32)
            nc.sync.dma_start(tx[:], x_d[:])

            # ---- GroupNorm statistics ----
            stats_f = small.tile([C, 2], F32)
            sq_scratch = data.tile([C, N], F32)
            nc.vector.tensor_reduce(
                stats_f[:, 0:1], tx[:], axis=mybir.AxisListType.X, op=OP.add
            )
            nc.scalar.activation(
                sq_scratch[:], tx[:], AF.Square, accum_out=stats_f[:, 1:2]
            )
            stats = small.tile([C, 2], F32R)
            nc.vector.tensor_copy(stats[:], stats_f[:])

            with tc.tile_pool(name="gnps", bufs=1, space="PSUM") as gnps:
                psg = gnps.tile([GROUPS, 2], F32)
                nc.tensor.matmul(psg[:], gmat[:], stats[:], start=True, stop=True)

                inv_n = 1.0 / (GSIZE * N)
                t32 = small.tile([GROUPS, 4], F32)
                nc.vector.tensor_scalar_mul(t32[:, 0:1], psg[:, 0:1], inv_n)
                nc.vector.tensor_scalar_mul(t32[:, 1:2], psg[:, 1:2], inv_n)
                nc.vector.tensor_mul(t32[:, 2:3], t32[:, 0:1], t32[:, 0:1])
                nc.vector.tensor_sub(t32[:, 3:4], t32[:, 1:2], t32[:, 2:3])
                eps_t = small.tile([GROUPS, 1], F32)
                nc.vector.memset(eps_t[:], EPS)
                nc.scalar.activation(t32[:, 2:3], t32[:, 3:4], AF.Sqrt, bias=eps_t[:])
                rstd_f = small.tile([GROUPS, 1], F32)
                nc.vector.reciprocal(rstd_f[:], t32[:, 2:3])
                mstat = small.tile([GROUPS, 2], F32R)
                nc.vector.tensor_copy(mstat[:, 0:1], t32[:, 0:1])
                nc.vector.tensor_copy(mstat[:, 1:2], rstd_f[:])

                pse = gnps.tile([C, 2], F32)
                nc.tensor.matmul(pse[:], gexp[:], mstat[:], start=True, stop=True)

                A_sb = small.tile([C, 1], F32)
                B_sb = small.tile([C, 1], F32)
                nc.vector.tensor_mul(A_sb[:], pse[:, 1:2], gnw[:])
                nc.vector.tensor_mul(B_sb[:], pse[:, 0:1], A_sb[:])
                nc.vector.tensor_sub(B_sb[:], gnb[:], B_sb[:])

            xn = data.tile([C, N], F32R)
            nc.scalar.activation(xn[:], tx[:], AF.Identity, bias=B_sb[:], scale=A_sb[:])

            # ---- Q/K/V projections (q written w-major) ----
            q = data.tile([C, N], F32R)
            k = data.tile([C, N], F32R)
            v = data.tile([C, N], BF16)
            q_wh = q[:].rearrange("p (w h) -> p h w", h=H)
            with tc.tile_pool(name="projps", bufs=2, space="PSUM") as projps:
                for wT, bias, dst, permute in (
                    (wqT, bq, q, True),
                    (wkT, bk, k, False),
                    (wvT, bv, v, False),
                ):
                    for g in range(NQG):
                        pp = projps.tile([C, QG], F32, tag="pp")
                        o = g * QG
                        nc.tensor.matmul(
                            pp[:, 0:512], wT[:], xn[:, o : o + 512],
                            start=True, stop=True,
                        )
                        nc.tensor.matmul(
                            pp[:, 512:QG], wT[:], xn[:, o + 512 : o + QG],
                            start=True, stop=True,
                        )
                        if permute:
                            outv = q_wh[:, 16 * g : 16 * (g + 1), :]
                            inv = pp[:].rearrange("p (h w) -> p h w", w=W)
                        else:
                            outv = dst[:, o : o + QG]
                            inv = pp[:, :]
                        nc.scalar.activation(outv, inv, AF.Identity, bias=bias[:])

                # v^T chunks (bf16) via PE transpose
                vT = data.tile([C, NCHUNK * C], BF16)
                for grp in range(0, NCHUNK, 4):
                    cnt = min(4, NCHUNK - grp)
                    pvt = projps.tile([C, 512], BF16, tag="pvt")
                    for j in range(cnt):
                        ch = grp + j
                        nc.tensor.transpose(
                            pvt[:, 128 * j : 128 * (j + 1)],
                            v[:, 128 * ch : 128 * (ch + 1)],
                            ident[:],
                        )
                    nc.scalar.copy(
                        vT[:, 128 * grp : 128 * (grp + cnt)], pvt[:, : 128 * cnt]
                    )

            # ---- main attention loop ----
            with (
                tc.tile_pool(name="ctxps", bufs=1, space="PSUM") as ctxps,
                tc.tile_pool(name="sps", bufs=1, space="PSUM") as sps,
            ):
                ctx_ps = [
                    ctxps.tile([C, sz], F32, tag=f"ctx{i}", name=f"ctx_ps{i}")
                    for i, sz in enumerate(CTX_SIZES)
                ]
                e_tiles = [None] * NCHUNK

                def emit_av(ch, part):
                    ec = e_tiles[ch]
                    banks = ([0, 1], [2, 3], [4])[part]
                    for i in banks:
                        o, sz = CTX_OFFS[i], CTX_SIZES[i]
                        nc.tensor.matmul(
                            ctx_ps[i][:, :],
                            vT[:, 128 * ch : 128 * (ch + 1)],
                            ec[:, o : o + sz],
                            start=(ch == 0),
                            stop=(ch == NCHUNK - 1),
                        )

                for it in range(NCHUNK + 2):
                    ch = it if it < NCHUNK else None
                    av = it - 2  # AV lagged two chunks: softmax chain is done
                    if ch is not None:
                        ec = epool.tile([C, N], BF16, tag="E", name=f"E_{ch}")
                        e_tiles[ch] = ec
                        klhs = k[:, 128 * ch : 128 * (ch + 1)]
                        for g in range(NQG):
                            ps = sps.tile([C, QG], F32, tag="spsum")
                            o = g * QG
                            nc.tensor.matmul(
                                ps[:, 0:512], klhs, q[:, o : o + 512],
                                start=True, stop=True,
                            )
                            nc.tensor.matmul(
                                ps[:, 512:QG], klhs, q[:, o + 512 : o + QG],
                                start=True, stop=True,
                            )
                            nc.scalar.activation(ec[:, o : o + QG], ps[:, :], AF.Exp)
                            if av >= 0:
                                emit_av(av, g)
                    else:
                        for g in range(NQG):
                            emit_av(av, g)

                    if ch is None:
                        continue
                    # softmax denominator + normalize
                    dsum = soft.tile([C, W], F32, tag="D")
                    nc.vector.tensor_reduce(
                        dsum[:],
                        ec[:].rearrange("p (w h) -> p w h", h=H),
                        axis=mybir.AxisListType.X,
                        op=OP.add,
                    )
                    rden = soft.tile([C, W], F32, tag="R")
                    nc.vector.reciprocal(rden[:], dsum[:])
                    ev = ec[:].rearrange("p (w h) -> p w h", h=H)
                    if ch % GP_MUL_EVERY == 1:
                        rden_b = soft.tile([C, W], BF16, tag="Rb")
                        nc.vector.tensor_copy(rden_b[:], rden[:])
                        nc.gpsimd.tensor_tensor(
                            out=ev, in0=ev,
                            in1=rden_b[:, :, None].to_broadcast([C, W, H]),
                            op=OP.mult,
                        )
                    else:
                        nc.vector.tensor_tensor(
                            out=ev, in0=ev,
                            in1=rden[:, :, None].to_broadcast([C, W, H]),
                            op=OP.mult,
                        )

                ctx_all = data.tile([C, N], F32R)
                for i, (o, sz) in enumerate(zip(CTX_OFFS, CTX_SIZES)):
                    nc.scalar.copy(ctx_all[:, o : o + sz], ctx_ps[i][:, :])

            # ---- output projection (back to natural order) + residual ----
            out_nat = data.tile([C, N], F32)
            out_wh = out_nat[:].rearrange("p (h w) -> p w h", w=W)
            with tc.tile_pool(name="ops", bufs=2, space="PSUM") as ops:
                for g in range(NQG):
                    po = ops.tile([C, QG], F32, tag="po")
                    o = g * QG
                    nc.tensor.matmul(
                        po[:, 0:512], woT[:], ctx_all[:, o : o + 512],
                        start=True, stop=True,
                    )
                    nc.tensor.matmul(
                        po[:, 512:QG], woT[:], ctx_all[:, o + 512 : o + QG],
                        start=True, stop=True,
                    )
                    nc.scalar.activation(
                        out_wh[:, 16 * g : 16 * (g + 1), :],
                        po[:].rearrange("p (w h) -> p w h", h=H),
                        AF.Identity,
                        bias=bo[:],
                    )
            nc.vector.tensor_add(out_nat[:], out_nat[:], tx[:])
            nc.sync.dma_start(out_d[:], out_nat[:])

    nc.compile()
    return nc


_PROGRAM_CACHE = None


def kernel(**inputs: np.ndarray) -> np.ndarray:
    global _PROGRAM_CACHE
    if _PROGRAM_CACHE is None:
        _PROGRAM_CACHE = _build_program()
    nc = _PROGRAM_CACHE

    import ml_dtypes

    f32 = lambda a: np.ascontiguousarray(np.asarray(a), dtype=np.float32)
    x = f32(inputs["x"])
    scale = 1.0 / np.sqrt(np.float32(C))

    gmat = np.zeros((C, GROUPS), np.float32)
    gmat[np.arange(C), np.arange(C) // GSIZE] = 1.0

    shared = {
        "gn_w": f32(inputs["gn_w"]).reshape(C, 1),
        "gn_b": f32(inputs["gn_b"]).reshape(C, 1),
        "wqT": np.ascontiguousarray(f32(inputs["wq"]).T * scale),
        "wkT": np.ascontiguousarray(f32(inputs["wk"]).T),
        "wvT": np.ascontiguousarray(f32(inputs["wv"]).T),
        "woT": np.ascontiguousarray(f32(inputs["wo"]).T),
        "bq": f32(inputs["bq"]).reshape(C, 1) * scale,
        "bk": f32(inputs["bk"]).reshape(C, 1),
        "bv": f32(inputs["bv"]).reshape(C, 1),
        "bo": f32(inputs["bo"]).reshape(C, 1),
        "gmat": gmat,
        "gexp": np.ascontiguousarray(gmat.T),
        "ident": np.eye(C).astype(ml_dtypes.bfloat16),
    }
    in_maps = [
        {**shared, "x": np.ascontiguousarray(x[b].reshape(C, N))} for b in range(B)
    ]

    res = bass_utils.run_bass_kernel_spmd(nc, in_maps, core_ids=list(range(NCORES)))
    out = np.stack([res.results[b]["out"].reshape(C, H, W) for b in range(B)])
    return out.astype(np.float32)
